# revision 18
# baseline (speedup 1.0000x reference)
# BERT self-attention with relation bias (Tableformer) on 8 TRN2 NeuronCores.
#
# Strategy (per core = one batch element, pure data parallelism over B=8):
#   - Q^T/K^T/V projections in bf16 on TensorE (inputs pre-transposed host-side,
#     which is pure layout marshalling; all arithmetic runs on-device).
#   - scores computed TRANSPOSED: S^T[k, q] = sum_d K^T[d,k] * Q^T[d,q] so the
#     attention-mask add and softmax plumbing use per-partition (k) bias slots.
#   - softmax without max-subtraction (scores are O(1) here).
#   - relation bias ADDED PRE-EXP: delta_rh = E[r,h] - E[6,h] (normalizing by
#     E[6,h] cancels in softmax).  Two UNCHAINED custom-DVE lookups build
#     Da = delta[rel] for rel in {0,1,2} and Db for rel in {3,4,5} (via a
#     shifted rel-3 plane); each is a single fused 3-entry lookup using the
#     C3 fourth-scalar latch.  The planes depend only on `rel`, NOT on the
#     scores, so the DVE runs flat-out from t~=10us with no upstream stalls.
#   - PE ADDS Da/Db into the scores PSUM via identity-matmul accumulation
#     (psum += I @ D), so no post-exp multiply pass exists at all; ACT's exp
#     reads the fully-biased psum and writes P^T (pt) directly.
#   - ctx^T via a second matmul with P^T as the stationary operand; the softmax
#     denominator comes from a ones-column appended to V (column 64 of V').
#   - final division by the row-sum via ACT Identity with a per-partition
#     reciprocal scale; per-head output slab DMA'd out as soon as the head
#     completes (no tail output flush).
#   - projections are INTERLEAVED into the attention head loop (K/Q block 0
#     first; later blocks + V fill PE gaps), so no serial projection prologue.
import os
import sys
import numpy as np

sys.path.insert(0, "/opt/trn_rl_repo")

import concourse.mybir as mybir  # noqa: E402
from concourse import bass, bacc, tile  # noqa: E402
from concourse.bass_utils import run_bass_kernel_spmd  # noqa: E402
from concourse.dve_ops import DveOp, OPS, CUSTOM_DVE_SPECS, get_dve_sub_opcode  # noqa: E402
from concourse.dve_spec import (  # noqa: E402
    Spec, Src0, Src1, C0, C1, C2, C3, One, Zero, select, eq, lower, _has_src1,
    _spill_c3_to_src1,
)
from concourse.dve_uop import DveOpSpec  # noqa: E402
from concourse.dve_table_gen import dve_ver_for  # noqa: E402

B, S, D, H, HD, NREL = 8, 1024, 1024, 16, 64, 7
N_CORES = 8
P = 128
NT = S // P  # 8 tiles along any 1024 dim
F32 = mybir.dt.float32
BF16 = mybir.dt.bfloat16
I32 = mybir.dt.int32
AF = mybir.ActivationFunctionType
OP = mybir.AluOpType

# ---------------------------------------------------------------------------
# Custom DVE op: out = (in0==0 ? s0 : in0==1 ? s1 : in0==imm2 ? latch(in1) : 0)
# A 3-entry table lookup; imm2 is 2.0 at every call site, the third table
# value rides the C3 latch (in1 = [P,1] column, read once at element 0).
# ---------------------------------------------------------------------------
_LUT3 = None


def _register_lut3():
    global _LUT3
    if _LUT3 is not None:
        return _LUT3
    for op in OPS:
        if op.name == "REL_LUT3":
            _LUT3 = op
            return op

    body = _spill_c3_to_src1(
        select(eq(Src0, Zero), C0, select(eq(Src0, One), C1, select(eq(Src0, C2), C3, Zero)))
    )

    def _ref(in0, in1, s0, s1, imm2):
        return np.where(
            in0 == 0, s0, np.where(in0 == 1, s1, np.where(in0 == imm2, in1, np.float32(0.0)))
        ).astype(np.float32)

    sp = Spec(body=body, reference=_ref)
    import concourse.dve_ops as _dvo

    op = DveOp("REL_LUT3", sp, subdim=False, uops_sha={})
    OPS.append(op)
    CUSTOM_DVE_SPECS[op.name] = sp
    _dvo._SUB_OPCODE_FOR_NAME[op.name] = _dvo._CUSTOM_DVE_ROW_BASE + len(OPS) - 1
    assert _dvo._SUB_OPCODE_FOR_NAME[op.name] < 0x20
    for ver in ("v3", "v4"):
        try:
            d = DveOpSpec(
                name=op.name,
                opcode=get_dve_sub_opcode(op.name),
                uops=lower(sp, ver=ver),
                rd1_en=_has_src1(sp),
            )
            op.uops_sha[ver] = d.sha(ver)
        except Exception:
            pass
    _LUT3 = op
    return op


# ---------------------------------------------------------------------------
# Program builder (runs once per process; input-value independent)
# ---------------------------------------------------------------------------
def _build_program():
    lut3 = _register_lut3()

    nc = bacc.Bacc(
        "TRN2",
        target_bir_lowering=False,
        debug=False,
        enable_asserts=False,
        num_devices=N_CORES,
    )

    # DRAM I/O (per core)
    xT_d = nc.dram_tensor("xT", [D, S], F32, kind="ExternalInput")       # hidden[b].T  [din, seq]
    wqT_d = nc.dram_tensor("wqT", [D, D], F32, kind="ExternalInput")     # Wq.T [din, dout]
    wkT_d = nc.dram_tensor("wkT", [D, D], F32, kind="ExternalInput")
    wvT_d = nc.dram_tensor("wvT", [D, D], F32, kind="ExternalInput")
    bq_d = nc.dram_tensor("bq", [D], F32, kind="ExternalInput")
    bk_d = nc.dram_tensor("bk", [D], F32, kind="ExternalInput")
    bv_d = nc.dram_tensor("bv", [D], F32, kind="ExternalInput")
    relT_d = nc.dram_tensor("relT", [S, S], I32, kind="ExternalInput")   # relation[b].T  [k, q]
    mask_d = nc.dram_tensor("maskv", [S], F32, kind="ExternalInput")     # attention_mask[b,0,0,:]
    remb_d = nc.dram_tensor("relemb", [NREL, H], F32, kind="ExternalInput")
    out_d = nc.dram_tensor("out", [S, D], F32, kind="ExternalOutput")

    from contextlib import ExitStack

    with tile.TileContext(nc) as tc, ExitStack() as ctx:
        const = ctx.enter_context(tc.tile_pool(name="const", bufs=1))

        # persistent SBUF tensors
        qT = const.tile([P, NT * S], BF16)       # Q^T/8 (+bq/8), dout on partitions
        kT = const.tile([P, NT * S], BF16)       # K^T  (+bk)
        vP = const.tile([P, NT * H * (HD + 1)], BF16)  # V' per seq-block: 16*(64+1)
        rel0 = const.tile([P, NT * S], BF16)     # rel^T as bf16
        rel3 = const.tile([P, NT * S], BF16)     # rel^T - 3
        xT = const.tile([P, NT * S], BF16)       # hidden^T as bf16
        wv = const.tile([P, NT * S], BF16)       # full Wv^T (rhs of V proj)
        mcols = const.tile([P, NT], F32)         # mask column per k-tile
        bqcols = const.tile([P, NT], F32)        # bq/8 column per dout-block
        bkcols = const.tile([P, NT], F32)
        mraw = const.tile([P, 6 * H], F32)       # delta_rh = E[r,h]-E[6,h], col r*16+h
        ident = const.tile([P, P], BF16)         # 128x128 identity (bias-add lhsT)
        ones_row = const.tile([1, P], F32)       # lhsT for broadcast matmul
        ones_row_bf = const.tile([1, P], BF16)   # bf16 lhsT for rank-1 bias matmul
        bv_row2 = const.tile([1, D], BF16)       # bv as a single-partition row

        with (
            tc.tile_pool(name="prep", bufs=1) as prep,
            tc.tile_pool(name="wrot", bufs=2) as wrot,        # rotating K/Q W-blocks
            tc.tile_pool(name="proj_ps", bufs=2, space="PSUM") as proj_ps,
            tc.tile_pool(name="pt", bufs=2) as ptp,
            tc.tile_pool(name="sc_ps", bufs=2, space="PSUM") as sc_psp,
            tc.tile_pool(name="cx_ps", bufs=2, space="PSUM") as cx_psp,
            tc.tile_pool(name="dlt", bufs=4) as dlt,          # Da/Db bias planes
            tc.tile_pool(name="oh", bufs=2) as ohp,           # per-head output slab
            tc.tile_pool(name="rc", bufs=2) as rcp,
        ):
            # ---------------- input DMAs (issue early; Pool dispatch ~1us each)
            def dma_w_block(i):
                """K/Q projection lhsT for dout-block i: W columns i*P..(i+1)*P,
                laid [P din-in-tile, kk*P + col] for the 8 din tiles."""
                wkb = wrot.tile([P, NT * P], BF16, tag="wk")
                wqb = wrot.tile([P, NT * P], BF16, tag="wq")
                nc.gpsimd.dma_start(
                    out=wkb[:].rearrange("p (t c) -> p t c", t=NT),
                    in_=wkT_d[:, i * P:(i + 1) * P].rearrange("(t p) c -> p t c", p=P),
                )
                nc.gpsimd.dma_start(
                    out=wqb[:].rearrange("p (t c) -> p t c", t=NT),
                    in_=wqT_d[:, i * P:(i + 1) * P].rearrange("(t p) c -> p t c", p=P),
                )
                return wkb, wqb

            w0 = dma_w_block(0)

            # bulk loads, 2 batched DMAs each (split for DMA-queue parallelism)
            for lo in (0, NT // 2):
                sl = slice(lo * P, (lo + NT // 2) * P)
                nc.gpsimd.dma_start(
                    out=xT[:, lo * S:(lo + NT // 2) * S].rearrange("p (t c) -> p t c", t=NT // 2),
                    in_=xT_d[sl, :].rearrange("(t p) c -> p t c", p=P),
                )
            # relation plane (DMA-cast int32 -> bf16; values 0..6 exact),
            # quartered so the first Delta lookups can start ASAP; the rel-3
            # shifted plane follows each quarter on Pool (values exact in bf16)
            for lo in (0, 2, 4, 6):
                sl = slice(lo * P, (lo + 2) * P)
                nc.gpsimd.dma_start(
                    out=rel0[:, lo * S:(lo + 2) * S].rearrange("p (t c) -> p t c", t=2),
                    in_=relT_d[sl, :].rearrange("(t p) c -> p t c", p=P),
                )
                nc.gpsimd.tensor_scalar_add(
                    rel3[:, lo * S:(lo + 2) * S],
                    rel0[:, lo * S:(lo + 2) * S],
                    -3.0,
                )
            for lo in (0, NT // 2):
                sl = slice(lo * P, (lo + NT // 2) * P)
                nc.gpsimd.dma_start(
                    out=wv[:, lo * S:(lo + NT // 2) * S].rearrange("p (t c) -> p t c", t=NT // 2),
                    in_=wvT_d[sl, :].rearrange("(t p) c -> p t c", p=P),
                )

            # ---------------- constants prep ----------------
            nc.sync.dma_start(out=mcols[:], in_=mask_d[:].rearrange("(t p) -> p t", p=P))
            nc.sync.dma_start(out=bqcols[:], in_=bq_d[:].rearrange("(t p) -> p t", p=P))
            nc.sync.dma_start(out=bkcols[:], in_=bk_d[:].rearrange("(t p) -> p t", p=P))
            nc.vector.tensor_scalar_mul(bqcols[:], bqcols[:], 0.125)

            nc.vector.memset(ones_row[:], 1.0)

            # rel_emb broadcast to all partitions: [1,112] -> psum [128,112]
            remb_row = prep.tile([1, NREL * H], F32)
            nc.sync.dma_start(
                out=remb_row[:], in_=remb_d[:].rearrange("r h -> (r h)").rearrange("(o n) -> o n", o=1)
            )
            mb_ps = proj_ps.tile([P, 512], F32, tag="pps")
            nc.tensor.matmul(mb_ps[:, 0:NREL * H], ones_row[:], remb_row[:])
            mb_sb = prep.tile([P, NREL * H], F32)
            nc.vector.tensor_copy(mb_sb[:], mb_ps[:, 0:NREL * H])
            # delta_r = E[r,:] - E[6,:] for r=0..5
            for r in range(6):
                nc.vector.tensor_tensor(
                    mraw[:, r * H:(r + 1) * H],
                    mb_sb[:, r * H:(r + 1) * H],
                    mb_sb[:, 6 * H:7 * H],
                    OP.subtract,
                )

            # identity matrix for the PSUM bias-add matmuls:
            # riota[p, c] = c ; piota[p, 0] = p ; I = (riota == piota)
            riota = prep.tile([P, P], F32)
            piota = prep.tile([P, 1], F32)
            nc.gpsimd.iota(riota[:], [[1, P]], channel_multiplier=0,
                           allow_small_or_imprecise_dtypes=True)
            nc.gpsimd.iota(piota[:], [[0, 1]], channel_multiplier=1,
                           allow_small_or_imprecise_dtypes=True)
            nc.vector.tensor_scalar(ident[:], riota[:], piota[:], None, OP.is_equal)

            nc.gpsimd.dma_start(out=bv_row2[:], in_=bv_d[:].rearrange("(o d) -> o d", o=1))
            nc.vector.memset(ones_row_bf[:], 1.0)

            # V' gets ones in column 64 of each head slot (denominator column)
            nc.gpsimd.memset(vP[:], 1.0)

            # ---------------- emission helpers ----------------
            def emit_kq_block(i, wpair):
                """Project K then Q for dout-block i (heads 2i, 2i+1)."""
                wkb, wqb = wpair
                for which in ("k", "q"):
                    wt = wkb if which == "k" else wqb
                    dst = kT if which == "k" else qT
                    bias_cols = bkcols if which == "k" else bqcols
                    scale = 1.0 if which == "k" else 0.125
                    for j in range(2):
                        ps = proj_ps.tile([P, 512], F32, tag="pps")
                        for kk in range(NT):
                            nc.tensor.matmul(
                                ps[:],
                                wt[:, kk * P:(kk + 1) * P],
                                xT[:, kk * S + j * 512: kk * S + (j + 1) * 512],
                                start=(kk == 0),
                                stop=(kk == NT - 1),
                            )
                        nc.scalar.activation(
                            dst[:, i * S + j * 512: i * S + (j + 1) * 512],
                            ps[:], AF.Identity,
                            bias=bias_cols[:, i:i + 1], scale=scale,
                        )

            def emit_v_block(sb):
                """Project V for seq-block sb into vP (natural layout + ones col)."""
                for j in range(2):
                    ps = proj_ps.tile([P, 512], F32, tag="pps")
                    for kk in range(NT):
                        nc.tensor.matmul(
                            ps[:],
                            xT[:, kk * S + sb * P: kk * S + (sb + 1) * P],
                            wv[:, kk * S + j * 512: kk * S + (j + 1) * 512],
                            start=(kk == 0),
                            stop=False,
                        )
                    # + bv via a rank-1 accumulating matmul (ones column x bv row)
                    nc.tensor.matmul(
                        ps[:],
                        ones_row_bf[:],
                        bv_row2[:, j * 512:(j + 1) * 512],
                        start=False,
                        stop=True,
                    )
                    vslot = vP[:, sb * H * 65 + j * 8 * 65: sb * H * 65 + (j + 1) * 8 * 65].rearrange(
                        "p (h e) -> p h e", h=8
                    )[:, :, 0:HD]
                    nc.scalar.activation(
                        vslot,
                        ps[:].rearrange("p (h e) -> p h e", h=8),
                        AF.Copy,
                    )

            def emit_ctx(h, pt, oh):
                for qb in range(NT):
                    cps = cx_psp.tile([P, HD + 1], F32, tag="cps")
                    for kb in range(NT):
                        nc.tensor.matmul(
                            cps[:],
                            pt[:, kb * S + qb * P: kb * S + (qb + 1) * P],
                            vP[:, kb * H * 65 + h * 65: kb * H * 65 + (h + 1) * 65],
                            start=(kb == 0),
                            stop=(kb == NT - 1),
                        )
                    rc = rcp.tile([P, 1], F32, tag="rc")
                    nc.vector.reciprocal(rc[:], cps[:, HD:HD + 1])
                    nc.scalar.activation(
                        oh[:, qb * HD:(qb + 1) * HD],
                        cps[:, 0:HD], AF.Identity, bias=0.0, scale=rc[:],
                    )
                # stream this head's output columns to DRAM
                nc.gpsimd.dma_start(
                    out=out_d[:, h * HD:(h + 1) * HD].rearrange("(t p) c -> p t c", p=P),
                    in_=oh[:].rearrange("p (t c) -> p t c", t=NT),
                )

            # ---------------- interleaved projections + attention ----------
            emit_kq_block(0, w0)

            prev = None
            wnext = None
            for h in range(H):
                off = (h % 2) * HD
                hc = h // 2
                # prefetch + project the next K/Q block one head-pair ahead
                if h % 2 == 0 and hc + 1 < NT:
                    wnext = dma_w_block(hc + 1)
                elif h % 2 == 1 and wnext is not None:
                    emit_kq_block(h // 2 + 1, wnext)
                    wnext = None

                pt = ptp.tile([P, NT * S], BF16, tag="pt")
                for kb2 in range(NT // 2):
                    # bias planes for 2 k-tiles: depend only on rel + rel_emb,
                    # so the DVE never waits on the score pipeline.
                    da = dlt.tile([P, 2 * S], BF16, tag="da")
                    db = dlt.tile([P, 2 * S], BF16, tag="db")
                    sl2 = slice(kb2 * 2 * S, (kb2 * 2 + 2) * S)
                    nc.vector._custom_dve(
                        lut3, out=da[:], in0=rel0[:, sl2],
                        in1=mraw[:, 2 * H + h: 2 * H + h + 1],
                        s0=mraw[:, 0 * H + h: 0 * H + h + 1],
                        s1=mraw[:, 1 * H + h: 1 * H + h + 1],
                        imm2=2.0,
                    )
                    nc.vector._custom_dve(
                        lut3, out=db[:], in0=rel3[:, sl2],
                        in1=mraw[:, 5 * H + h: 5 * H + h + 1],
                        s0=mraw[:, 3 * H + h: 3 * H + h + 1],
                        s1=mraw[:, 4 * H + h: 4 * H + h + 1],
                        imm2=2.0,
                    )
                    for kh in range(2):
                        kb = kb2 * 2 + kh
                        ps = sc_psp.tile([P, S], F32, tag="scps")
                        for j in range(2):
                            nc.tensor.matmul(
                                ps[:, j * 512:(j + 1) * 512],
                                kT[off:off + HD, hc * S + kb * P: hc * S + (kb + 1) * P],
                                qT[off:off + HD, hc * S + j * 512: hc * S + (j + 1) * 512],
                                start=True, stop=False,
                            )
                            nc.tensor.matmul(
                                ps[:, j * 512:(j + 1) * 512],
                                ident[:],
                                da[:, kh * S + j * 512: kh * S + (j + 1) * 512],
                                start=False, stop=False,
                            )
                            nc.tensor.matmul(
                                ps[:, j * 512:(j + 1) * 512],
                                ident[:],
                                db[:, kh * S + j * 512: kh * S + (j + 1) * 512],
                                start=False, stop=True,
                            )
                        nc.scalar.activation(
                            pt[:, kb * S:(kb + 1) * S], ps[:], AF.Exp,
                            bias=mcols[:, kb:kb + 1], scale=1.0,
                        )

                # V projections fill the PE gaps during the first two heads;
                # all 8 blocks are emitted before ctx(0).
                if h == 0:
                    for sb in range(4):
                        emit_v_block(sb)
                elif h == 1:
                    for sb in range(4, NT):
                        emit_v_block(sb)

                # ctx pipelined one head behind: PE emits scores(h+1)
                # before ctx(h) would otherwise block it.
                if prev is not None:
                    emit_ctx(*prev)
                oh = ohp.tile([P, NT * HD], BF16, tag="oh")
                prev = (h, pt, oh)

            if prev is not None:
                emit_ctx(*prev)

    nc.compile()
    return nc


_PROGRAM = None


def _get_program():
    global _PROGRAM
    if _PROGRAM is None:
        _PROGRAM = _build_program()
    return _PROGRAM


def _make_in_maps(inputs):
    hidden = np.asarray(inputs["hidden_states"], dtype=np.float32)
    mask = np.asarray(inputs["attention_mask"], dtype=np.float32)
    relation = np.asarray(inputs["relation"], dtype=np.int32)
    wq = np.ascontiguousarray(np.asarray(inputs["Wq"], dtype=np.float32).T)
    wk = np.ascontiguousarray(np.asarray(inputs["Wk"], dtype=np.float32).T)
    wv = np.ascontiguousarray(np.asarray(inputs["Wv"], dtype=np.float32).T)
    bq = np.asarray(inputs["bq"], dtype=np.float32)
    bk = np.asarray(inputs["bk"], dtype=np.float32)
    bv = np.asarray(inputs["bv"], dtype=np.float32)
    remb = np.asarray(inputs["rel_emb"], dtype=np.float32)

    in_maps = []
    for b in range(N_CORES):
        in_maps.append({
            "xT": np.ascontiguousarray(hidden[b].T),
            "wqT": wq, "wkT": wk, "wvT": wv,
            "bq": bq, "bk": bk, "bv": bv,
            "relT": np.ascontiguousarray(relation[b].T),
            "maskv": np.ascontiguousarray(mask[b, 0, 0, :]),
            "relemb": remb,
        })
    return in_maps


LAST_EXEC_NS = None
LAST_RESULTS = None


def kernel(**inputs) -> np.ndarray:
    global LAST_EXEC_NS, LAST_RESULTS
    nc = _get_program()
    in_maps = _make_in_maps(inputs)
    trace = os.environ.get("KERNEL_TRACE", "0") == "1"
    res = run_bass_kernel_spmd(nc, in_maps, list(range(N_CORES)), trace=trace)
    LAST_EXEC_NS = res.exec_time_ns
    LAST_RESULTS = res
    out = np.stack([res.results[b]["out"] for b in range(N_CORES)], axis=0)
    return out.astype(np.float32)


# -------- timing helper: device-resident repeated dispatch --------
def make_bench_fn(inputs):
    """Returns run(M) -> seconds for M back-to-back dispatches (device-resident
    inputs, no donation, block at the end)."""
    import jax
    from jax.sharding import Mesh, PartitionSpec, NamedSharding
    from jax.experimental.shard_map import shard_map
    from concourse import bass2jax
    import concourse.mybir as mb

    nc = _get_program()
    in_maps = _make_in_maps(inputs)
    bass2jax.install_neuronx_cc_hook()

    part_name = nc.partition_id_tensor.name if nc.partition_id_tensor else None
    in_names, out_names, out_avals, zero_outs = [], [], [], []
    for alloc in nc.m.functions[0].allocations:
        if not isinstance(alloc, mb.MemoryLocationSet):
            continue
        name = alloc.memorylocations[0].name
        if alloc.kind == "ExternalInput":
            if name != part_name:
                in_names.append(name)
        elif alloc.kind == "ExternalOutput":
            out_names.append(name)
            shape = tuple(alloc.tensor_shape)
            dtype = mb.dt.np(alloc.dtype)
            out_avals.append(jax.core.ShapedArray(shape, dtype))
            zero_outs.append(np.zeros(shape, dtype))
    n_params = len(in_names)
    all_names = in_names + out_names
    if part_name is not None:
        all_names.append(part_name)

    def _body(*args):
        operands = list(args)
        if part_name is not None:
            operands.append(bass2jax.partition_id_tensor())
        outs = bass2jax._bass_exec_p.bind(
            *operands,
            out_avals=tuple(out_avals),
            in_names=tuple(all_names),
            out_names=tuple(out_names),
            lowering_input_output_aliases=(),
            sim_require_finite=True,
            sim_require_nnan=True,
            nc=nc,
        )
        return tuple(outs)

    devices = jax.devices()[:N_CORES]
    mesh = Mesh(np.asarray(devices), ("core",))
    n_all = n_params + len(out_names)
    sharded = jax.jit(
        shard_map(
            _body, mesh=mesh,
            in_specs=(PartitionSpec("core"),) * n_all,
            out_specs=(PartitionSpec("core"),) * len(out_names),
            check_rep=False,
        ),
        keep_unused=True,
    )
    sh = NamedSharding(mesh, PartitionSpec("core"))
    concat_in = [
        jax.device_put(
            np.concatenate([np.asarray(in_maps[c][nm]) for c in range(N_CORES)], axis=0), sh
        )
        for nm in in_names
    ]
    concat_zeros = [
        jax.device_put(np.zeros((N_CORES * z.shape[0], *z.shape[1:]), z.dtype), sh)
        for z in zero_outs
    ]
    # warmup + compile
    out = sharded(*concat_in, *concat_zeros)
    jax.block_until_ready(out)

    import time

    def run(M):
        t0 = time.perf_counter()
        outs = None
        for _ in range(M):
            outs = sharded(*concat_in, *concat_zeros)
        jax.block_until_ready(outs)
        return time.perf_counter() - t0

    def get_out():
        outs = sharded(*concat_in, *concat_zeros)
        o = np.asarray(outs[0]).reshape(N_CORES, *out_avals[0].shape)
        return o

    run.get_out = get_out
    return run


# -------- simulation helper (single core) for test.py --------
def run_sim_core0(inputs):
    from concourse.bass_interp import CoreSim

    nc = _get_program()
    in_maps = _make_in_maps(inputs)
    sim = CoreSim(nc, trace=False)
    for k, v in in_maps[0].items():
        sim.tensor(k)[:] = v
    sim.simulate(check_with_hw=False)
    return np.array(sim.tensor("out"))


# revision 36
# speedup vs baseline: 1.5522x; 1.5522x over previous
# BERT self-attention with relation bias (Tableformer) on 8 TRN2 NeuronCores.
#
# Strategy (per core = one batch element, pure data parallelism over B=8):
#   - Q^T/K^T/V projections in bf16 on TensorE (inputs pre-transposed host-side,
#     which is pure layout marshalling; all arithmetic runs on-device).
#   - scores computed TRANSPOSED: S^T[k, q] = sum_d K^T[d,k] * Q^T[d,q] so the
#     attention-mask add and softmax plumbing use per-partition (k) bias slots.
#   - softmax without max-subtraction (scores are O(1) here).
#   - relation bias ADDED PRE-EXP: delta_rh = E[r,h] - E[6,h] (normalizing by
#     E[6,h] cancels in softmax).  Two UNCHAINED custom-DVE lookups build
#     Da = delta[rel] for rel in {0,1,2} and Db for rel in {3,4,5} (via a
#     shifted rel-3 plane); each is a single fused 3-entry lookup using the
#     C3 fourth-scalar latch.  The planes depend only on `rel`, NOT on the
#     scores, so the DVE runs flat-out from t~=10us with no upstream stalls.
#   - PE ADDS Da/Db into the scores PSUM via identity-matmul accumulation
#     (psum += I @ D), so no post-exp multiply pass exists at all; ACT's exp
#     reads the fully-biased psum and writes P^T (pt) directly.
#   - ctx^T via a second matmul with P^T as the stationary operand; the softmax
#     denominator comes from a ones-column appended to V (column 64 of V').
#   - final division by the row-sum via ACT Identity with a per-partition
#     reciprocal scale; per-head output slab DMA'd out as soon as the head
#     completes (no tail output flush).
#   - projections are INTERLEAVED into the attention head loop (K/Q block 0
#     first; later blocks + V fill PE gaps), so no serial projection prologue.
import os
import sys
import numpy as np

sys.path.insert(0, "/opt/trn_rl_repo")

import concourse.mybir as mybir  # noqa: E402
from concourse import bass, bacc, tile  # noqa: E402
from concourse.bass_utils import run_bass_kernel_spmd  # noqa: E402
from concourse.dve_ops import DveOp, OPS, CUSTOM_DVE_SPECS, get_dve_sub_opcode  # noqa: E402
from concourse.dve_spec import (  # noqa: E402
    Spec, Src0, Src1, C0, C1, C2, C3, One, Zero, select, eq, lower, _has_src1,
    _spill_c3_to_src1,
)
from concourse.dve_uop import DveOpSpec  # noqa: E402
from concourse.dve_table_gen import dve_ver_for  # noqa: E402

B, S, D, H, HD, NREL = 8, 1024, 1024, 16, 64, 7
N_CORES = 8
P = 128
NT = S // P  # 8 tiles along any 1024 dim
F32 = mybir.dt.float32
BF16 = mybir.dt.bfloat16
I32 = mybir.dt.int32
AF = mybir.ActivationFunctionType
OP = mybir.AluOpType

# ---------------------------------------------------------------------------
# Custom DVE ops.
#   REL_LUT3:  out = (in0==0 ? s0 : in0==1 ? s1 : in0==imm2 ? latch(in1) : 0)
#     A 3-entry additive table lookup; imm2 is 2.0 at every call site, the
#     third value rides the C3 latch (in1 = [P,1] column, read at element 0).
#   REL_LUT2_MUL / REL_LUT2H_MUL / REL_LUT45_MUL:
#     out = (in0==a ? s0 : in0==b ? s1 : 1) * in1 for (a,b) in
#     {(0,1),(2,3),(4,5)} — the chained multiplicative ladder, used for the
#     first two heads where PE is saturated with V/KQ projections.
# ---------------------------------------------------------------------------
_OPS_CACHE = None


def _pin_and_register(op, sp):
    import concourse.dve_ops as _dvo
    OPS.append(op)
    CUSTOM_DVE_SPECS[op.name] = sp
    _dvo._SUB_OPCODE_FOR_NAME[op.name] = _dvo._CUSTOM_DVE_ROW_BASE + len(OPS) - 1
    assert _dvo._SUB_OPCODE_FOR_NAME[op.name] < 0x20
    for ver in ("v3", "v4"):
        try:
            d = DveOpSpec(
                name=op.name,
                opcode=get_dve_sub_opcode(op.name),
                uops=lower(sp, ver=ver),
                rd1_en=_has_src1(sp),
            )
            op.uops_sha[ver] = d.sha(ver)
        except Exception:
            pass


def _register_ops():
    global _OPS_CACHE
    if _OPS_CACHE is not None:
        return _OPS_CACHE
    existing = {op.name: op for op in OPS}
    if "REL_LUT3" in existing:
        _OPS_CACHE = (
            existing["REL_LUT3"], existing["REL_LUT2_MUL"],
            existing["REL_LUT2H_MUL"], existing["REL_LUT45_MUL"],
        )
        return _OPS_CACHE

    body = _spill_c3_to_src1(
        select(eq(Src0, Zero), C0, select(eq(Src0, One), C1, select(eq(Src0, C2), C3, Zero)))
    )

    def _ref3(in0, in1, s0, s1, imm2):
        return np.where(
            in0 == 0, s0, np.where(in0 == 1, s1, np.where(in0 == imm2, in1, np.float32(0.0)))
        ).astype(np.float32)

    sp3 = Spec(body=body, reference=_ref3)
    lut3 = DveOp("REL_LUT3", sp3, subdim=False, uops_sha={})
    _pin_and_register(lut3, sp3)

    two = One + One
    three = two + One
    four = two + two
    five = four + One

    def mk(name, ca, cb, va, vb):
        b = select(eq(Src0, ca), C0, select(eq(Src0, cb), C1, One)) * Src1

        def _ref(in0, in1, s0, s1, imm2, _va=va, _vb=vb):
            return (
                np.where(in0 == _va, s0, np.where(in0 == _vb, s1, np.float32(1.0)))
                * in1
            )

        return name, Spec(body=b, reference=_ref)

    muls = []
    for name, sp in (
        mk("REL_LUT2_MUL", Zero, One, 0, 1),
        mk("REL_LUT2H_MUL", two, three, 2, 3),
        mk("REL_LUT45_MUL", four, five, 4, 5),
    ):
        op = DveOp(name, sp, subdim=False, uops_sha={})
        _pin_and_register(op, sp)
        muls.append(op)

    _OPS_CACHE = (lut3, *muls)
    return _OPS_CACHE


# ---------------------------------------------------------------------------
# Program builder (runs once per process; input-value independent)
# ---------------------------------------------------------------------------
def _build_program():
    lut3, lut01, lut23, lut45 = _register_ops()

    nc = bacc.Bacc(
        "TRN2",
        target_bir_lowering=False,
        debug=False,
        enable_asserts=False,
        num_devices=N_CORES,
    )

    # DRAM I/O (per core)
    xT_d = nc.dram_tensor("xT", [D, S], F32, kind="ExternalInput")       # hidden[b].T  [din, seq]
    wqT_d = nc.dram_tensor("wqT", [D, D], F32, kind="ExternalInput")     # Wq.T [din, dout]
    wkT_d = nc.dram_tensor("wkT", [D, D], F32, kind="ExternalInput")
    wvT_d = nc.dram_tensor("wvT", [D, D], F32, kind="ExternalInput")
    bq_d = nc.dram_tensor("bq", [D], F32, kind="ExternalInput")
    bk_d = nc.dram_tensor("bk", [D], F32, kind="ExternalInput")
    bv_d = nc.dram_tensor("bv", [D], F32, kind="ExternalInput")
    relT_d = nc.dram_tensor("relT", [S, S], I32, kind="ExternalInput")   # relation[b].T  [k, q]
    mask_d = nc.dram_tensor("maskv", [S], F32, kind="ExternalInput")     # attention_mask[b,0,0,:]
    remb_d = nc.dram_tensor("relemb", [NREL, H], F32, kind="ExternalInput")
    out_d = nc.dram_tensor("out", [S, D], F32, kind="ExternalOutput")

    from contextlib import ExitStack

    with tile.TileContext(nc) as tc, ExitStack() as ctx:
        const = ctx.enter_context(tc.tile_pool(name="const", bufs=1))

        # persistent SBUF tensors
        qT = const.tile([P, NT * S], BF16)       # Q^T/8 (+bq/8), dout on partitions
        kT = const.tile([P, NT * S], BF16)       # K^T  (+bk)
        vP = const.tile([P, NT * H * (HD + 1)], BF16)  # V' per seq-block: 16*(64+1)
        rel0 = const.tile([P, NT * S], BF16)     # rel^T as bf16
        rel3 = const.tile([P, NT * S], BF16)     # rel^T - 3
        xT = const.tile([P, NT * S], BF16)       # hidden^T as bf16
        wv = const.tile([P, NT * S], BF16)       # full Wv^T (rhs of V proj)
        mcols = const.tile([P, NT], F32)         # mask column per k-tile
        bqcols = const.tile([P, NT], F32)        # bq/8 column per dout-block
        bkcols = const.tile([P, NT], F32)
        mraw = const.tile([P, 6 * H], F32)       # delta_rh = E[r,h]-E[6,h], col r*16+h
        mprime = const.tile([P, 6 * H], F32)     # exp(delta_rh), for the ladder heads
        ident = const.tile([P, P], BF16)         # 128x128 identity (bias-add lhsT)
        ones_row = const.tile([1, P], F32)       # lhsT for broadcast matmul
        ones_row_bf = const.tile([1, P], BF16)   # bf16 lhsT for rank-1 bias matmul
        bv_row2 = const.tile([1, D], BF16)       # bv as a single-partition row

        with (
            tc.tile_pool(name="prep", bufs=1) as prep,
            tc.tile_pool(name="wrot", bufs=2) as wrot,        # rotating K/Q W-blocks
            tc.tile_pool(name="proj_ps", bufs=2, space="PSUM") as proj_ps,
            tc.tile_pool(name="pt", bufs=2) as ptp,
            tc.tile_pool(name="ex", bufs=2) as exp_pool,
            tc.tile_pool(name="sc_ps", bufs=2, space="PSUM") as sc_psp,
            tc.tile_pool(name="cx_ps", bufs=2, space="PSUM") as cx_psp,
            tc.tile_pool(name="dlt", bufs=4) as dlt,          # Da/Db bias planes
            tc.tile_pool(name="oh", bufs=2) as ohp,           # per-head output slab
            tc.tile_pool(name="rc", bufs=2) as rcp,
        ):
            # ---------------- input DMAs (issue early; Pool dispatch ~1us each)
            def dma_w_block(i):
                """K/Q projection lhsT for dout-block i: W columns i*P..(i+1)*P,
                laid [P din-in-tile, kk*P + col] for the 8 din tiles."""
                wkb = wrot.tile([P, NT * P], BF16, tag="wk")
                wqb = wrot.tile([P, NT * P], BF16, tag="wq")
                nc.gpsimd.dma_start(
                    out=wkb[:].rearrange("p (t c) -> p t c", t=NT),
                    in_=wkT_d[:, i * P:(i + 1) * P].rearrange("(t p) c -> p t c", p=P),
                )
                nc.gpsimd.dma_start(
                    out=wqb[:].rearrange("p (t c) -> p t c", t=NT),
                    in_=wqT_d[:, i * P:(i + 1) * P].rearrange("(t p) c -> p t c", p=P),
                )
                return wkb, wqb

            w0 = dma_w_block(0)

            # bulk loads, 2 batched DMAs each (split for DMA-queue parallelism)
            for lo in (0, NT // 2):
                sl = slice(lo * P, (lo + NT // 2) * P)
                nc.gpsimd.dma_start(
                    out=xT[:, lo * S:(lo + NT // 2) * S].rearrange("p (t c) -> p t c", t=NT // 2),
                    in_=xT_d[sl, :].rearrange("(t p) c -> p t c", p=P),
                )
            # relation plane (DMA-cast int32 -> bf16; values 0..6 exact),
            # quartered so the first Delta lookups can start ASAP; the rel-3
            # shifted plane follows each quarter on Pool (values exact in bf16)
            for lo in (0, 2, 4, 6):
                sl = slice(lo * P, (lo + 2) * P)
                nc.gpsimd.dma_start(
                    out=rel0[:, lo * S:(lo + 2) * S].rearrange("p (t c) -> p t c", t=2),
                    in_=relT_d[sl, :].rearrange("(t p) c -> p t c", p=P),
                )
                nc.gpsimd.tensor_scalar_add(
                    rel3[:, lo * S:(lo + 2) * S],
                    rel0[:, lo * S:(lo + 2) * S],
                    -3.0,
                )
            for lo in (0, NT // 2):
                sl = slice(lo * P, (lo + NT // 2) * P)
                nc.gpsimd.dma_start(
                    out=wv[:, lo * S:(lo + NT // 2) * S].rearrange("p (t c) -> p t c", t=NT // 2),
                    in_=wvT_d[sl, :].rearrange("(t p) c -> p t c", p=P),
                )

            # ---------------- constants prep ----------------
            nc.sync.dma_start(out=mcols[:], in_=mask_d[:].rearrange("(t p) -> p t", p=P))
            nc.sync.dma_start(out=bqcols[:], in_=bq_d[:].rearrange("(t p) -> p t", p=P))
            nc.sync.dma_start(out=bkcols[:], in_=bk_d[:].rearrange("(t p) -> p t", p=P))
            nc.vector.tensor_scalar_mul(bqcols[:], bqcols[:], 0.125)

            nc.vector.memset(ones_row[:], 1.0)

            # rel_emb broadcast to all partitions: [1,112] -> psum [128,112]
            remb_row = prep.tile([1, NREL * H], F32)
            nc.sync.dma_start(
                out=remb_row[:], in_=remb_d[:].rearrange("r h -> (r h)").rearrange("(o n) -> o n", o=1)
            )
            mb_ps = proj_ps.tile([P, 512], F32, tag="pps")
            nc.tensor.matmul(mb_ps[:, 0:NREL * H], ones_row[:], remb_row[:])
            mb_sb = prep.tile([P, NREL * H], F32)
            nc.vector.tensor_copy(mb_sb[:], mb_ps[:, 0:NREL * H])
            # delta_r = E[r,:] - E[6,:] for r=0..5
            for r in range(6):
                nc.vector.tensor_tensor(
                    mraw[:, r * H:(r + 1) * H],
                    mb_sb[:, r * H:(r + 1) * H],
                    mb_sb[:, 6 * H:7 * H],
                    OP.subtract,
                )
            nc.scalar.activation(mprime[:], mraw[:], AF.Exp)

            # identity matrix for the PSUM bias-add matmuls:
            # riota[p, c] = c ; piota[p, 0] = p ; I = (riota == piota)
            riota = prep.tile([P, P], F32)
            piota = prep.tile([P, 1], F32)
            nc.gpsimd.iota(riota[:], [[1, P]], channel_multiplier=0,
                           allow_small_or_imprecise_dtypes=True)
            nc.gpsimd.iota(piota[:], [[0, 1]], channel_multiplier=1,
                           allow_small_or_imprecise_dtypes=True)
            nc.vector.tensor_scalar(ident[:], riota[:], piota[:], None, OP.is_equal)

            nc.gpsimd.dma_start(out=bv_row2[:], in_=bv_d[:].rearrange("(o d) -> o d", o=1))
            nc.vector.memset(ones_row_bf[:], 1.0)

            # V' gets ones in column 64 of each head slot (denominator column)
            nc.gpsimd.memset(vP[:], 1.0)

            # ---------------- emission helpers ----------------
            def emit_kq_block(i, wpair):
                """Project K then Q for dout-block i (heads 2i, 2i+1)."""
                wkb, wqb = wpair
                for which in ("k", "q"):
                    wt = wkb if which == "k" else wqb
                    dst = kT if which == "k" else qT
                    bias_cols = bkcols if which == "k" else bqcols
                    scale = 1.0 if which == "k" else 0.125
                    for j in range(2):
                        ps = proj_ps.tile([P, 512], F32, tag="pps")
                        for kk in range(NT):
                            nc.tensor.matmul(
                                ps[:],
                                wt[:, kk * P:(kk + 1) * P],
                                xT[:, kk * S + j * 512: kk * S + (j + 1) * 512],
                                start=(kk == 0),
                                stop=(kk == NT - 1),
                            )
                        nc.scalar.activation(
                            dst[:, i * S + j * 512: i * S + (j + 1) * 512],
                            ps[:], AF.Identity,
                            bias=bias_cols[:, i:i + 1], scale=scale,
                        )

            def emit_v_block(sb):
                """Project V for seq-block sb into vP (natural layout + ones col)."""
                for j in range(2):
                    ps = proj_ps.tile([P, 512], F32, tag="pps")
                    for kk in range(NT):
                        nc.tensor.matmul(
                            ps[:],
                            xT[:, kk * S + sb * P: kk * S + (sb + 1) * P],
                            wv[:, kk * S + j * 512: kk * S + (j + 1) * 512],
                            start=(kk == 0),
                            stop=False,
                        )
                    # + bv via a rank-1 accumulating matmul (ones column x bv row)
                    nc.tensor.matmul(
                        ps[:],
                        ones_row_bf[:],
                        bv_row2[:, j * 512:(j + 1) * 512],
                        start=False,
                        stop=True,
                    )
                    vslot = vP[:, sb * H * 65 + j * 8 * 65: sb * H * 65 + (j + 1) * 8 * 65].rearrange(
                        "p (h e) -> p h e", h=8
                    )[:, :, 0:HD]
                    nc.scalar.activation(
                        vslot,
                        ps[:].rearrange("p (h e) -> p h e", h=8),
                        AF.Copy,
                    )

            def emit_ctx(h, pt):
                oh = ohp.tile([P, NT * HD], BF16, tag="oh")
                for qb in range(NT):
                    cps = cx_psp.tile([P, HD + 1], F32, tag="cps")
                    for kb in range(NT):
                        nc.tensor.matmul(
                            cps[:],
                            pt[:, kb * S + qb * P: kb * S + (qb + 1) * P],
                            vP[:, kb * H * 65 + h * 65: kb * H * 65 + (h + 1) * 65],
                            start=(kb == 0),
                            stop=(kb == NT - 1),
                        )
                    rc = rcp.tile([P, 1], F32, tag="rc")
                    nc.vector.reciprocal(rc[:], cps[:, HD:HD + 1])
                    nc.scalar.activation(
                        oh[:, qb * HD:(qb + 1) * HD],
                        cps[:, 0:HD], AF.Identity, bias=0.0, scale=rc[:],
                    )
                # stream this head's output columns to DRAM
                nc.gpsimd.dma_start(
                    out=out_d[:, h * HD:(h + 1) * HD].rearrange("(t p) c -> p t c", p=P),
                    in_=oh[:].rearrange("p (t c) -> p t c", t=NT),
                )

            # ---------------- interleaved projections + attention ----------
            emit_kq_block(0, w0)

            N_LADDER = 2  # heads on the post-exp multiplicative-ladder path

            def emit_delta(h, kb2):
                """Bias planes for 2 k-tiles: depend only on rel + rel_emb, so
                the DVE never waits on the score pipeline."""
                da = dlt.tile([P, 2 * S], BF16, tag="da")
                db = dlt.tile([P, 2 * S], BF16, tag="db")
                sl2 = slice(kb2 * 2 * S, (kb2 * 2 + 2) * S)
                nc.vector._custom_dve(
                    lut3, out=da[:], in0=rel0[:, sl2],
                    in1=mraw[:, 2 * H + h: 2 * H + h + 1],
                    s0=mraw[:, 0 * H + h: 0 * H + h + 1],
                    s1=mraw[:, 1 * H + h: 1 * H + h + 1],
                    imm2=2.0,
                )
                nc.vector._custom_dve(
                    lut3, out=db[:], in0=rel3[:, sl2],
                    in1=mraw[:, 5 * H + h: 5 * H + h + 1],
                    s0=mraw[:, 3 * H + h: 3 * H + h + 1],
                    s1=mraw[:, 4 * H + h: 4 * H + h + 1],
                    imm2=2.0,
                )
                return da, db

            # head N_LADDER's bias planes are emitted up front so the DVE
            # starts producing them the moment rel0/rel3/mraw land.
            stash = [emit_delta(N_LADDER, kb2) for kb2 in range(NT // 2)]

            pend = []
            wnext = None
            for h in range(H):
                off = (h % 2) * HD
                hc = h // 2
                # prefetch + project the next K/Q block one head-pair ahead
                if h % 2 == 0 and hc + 1 < NT:
                    wnext = dma_w_block(hc + 1)
                elif h % 2 == 1 and wnext is not None:
                    emit_kq_block(h // 2 + 1, wnext)
                    wnext = None

                pt = ptp.tile([P, NT * S], BF16, tag="pt")
                if h < N_LADDER:
                    # ladder path: plain scores, exp, then 3 chained
                    # lookup-multiply ops.  No identity matmuls — PE is
                    # saturated with V/KQ projections during these heads,
                    # while the DVE would otherwise sit idle.
                    for kb2 in range(NT // 2):
                        ex = exp_pool.tile([P, 2 * S], BF16, tag="ex")
                        for kh in range(2):
                            kb = kb2 * 2 + kh
                            ps = sc_psp.tile([P, S], F32, tag="scps")
                            for j in range(2):
                                nc.tensor.matmul(
                                    ps[:, j * 512:(j + 1) * 512],
                                    kT[off:off + HD, hc * S + kb * P: hc * S + (kb + 1) * P],
                                    qT[off:off + HD, hc * S + j * 512: hc * S + (j + 1) * 512],
                                )
                            nc.scalar.activation(
                                ex[:, kh * S:(kh + 1) * S], ps[:], AF.Exp,
                                bias=mcols[:, kb:kb + 1], scale=1.0,
                            )
                        sl2 = slice(kb2 * 2 * S, (kb2 * 2 + 2) * S)
                        t1 = exp_pool.tile([P, 2 * S], BF16, tag="t1", bufs=1)
                        t2 = exp_pool.tile([P, 2 * S], BF16, tag="t2", bufs=1)
                        nc.vector._custom_dve(
                            lut01, out=t1[:], in0=rel0[:, sl2], in1=ex[:],
                            s0=mprime[:, 0 * H + h: 0 * H + h + 1],
                            s1=mprime[:, 1 * H + h: 1 * H + h + 1],
                        )
                        nc.vector._custom_dve(
                            lut23, out=t2[:], in0=rel0[:, sl2], in1=t1[:],
                            s0=mprime[:, 2 * H + h: 2 * H + h + 1],
                            s1=mprime[:, 3 * H + h: 3 * H + h + 1],
                        )
                        nc.vector._custom_dve(
                            lut45, out=pt[:, sl2], in0=rel0[:, sl2], in1=t2[:],
                            s0=mprime[:, 4 * H + h: 4 * H + h + 1],
                            s1=mprime[:, 5 * H + h: 5 * H + h + 1],
                        )
                else:
                    for kb2 in range(NT // 2):
                        if h == N_LADDER:
                            da, db = stash[kb2]
                        else:
                            da, db = emit_delta(h, kb2)
                        for kh in range(2):
                            kb = kb2 * 2 + kh
                            ps = sc_psp.tile([P, S], F32, tag="scps")
                            for j in range(2):
                                nc.tensor.matmul(
                                    ps[:, j * 512:(j + 1) * 512],
                                    kT[off:off + HD, hc * S + kb * P: hc * S + (kb + 1) * P],
                                    qT[off:off + HD, hc * S + j * 512: hc * S + (j + 1) * 512],
                                    start=True, stop=False,
                                )
                                nc.tensor.matmul(
                                    ps[:, j * 512:(j + 1) * 512],
                                    ident[:],
                                    da[:, kh * S + j * 512: kh * S + (j + 1) * 512],
                                    start=False, stop=False,
                                )
                                nc.tensor.matmul(
                                    ps[:, j * 512:(j + 1) * 512],
                                    ident[:],
                                    db[:, kh * S + j * 512: kh * S + (j + 1) * 512],
                                    start=False, stop=True,
                                )
                            nc.scalar.activation(
                                pt[:, kb * S:(kb + 1) * S], ps[:], AF.Exp,
                                bias=mcols[:, kb:kb + 1], scale=1.0,
                            )

                # V projections fill the PE gaps during the first two heads;
                # all 8 blocks are emitted before ctx(0).
                if h == 0:
                    for sb in range(4):
                        emit_v_block(sb)
                elif h == 1:
                    for sb in range(4, NT):
                        emit_v_block(sb)

                # ctx pipelined one head behind: PE emits scores(h+1)
                # before ctx(h) would otherwise block it.
                pend.append((h, pt))
                if len(pend) > 1:
                    emit_ctx(*pend.pop(0))

            for e in pend:
                emit_ctx(*e)

    nc.compile()
    return nc


_PROGRAM = None


def _get_program():
    global _PROGRAM
    if _PROGRAM is None:
        _PROGRAM = _build_program()
    return _PROGRAM


def _make_in_maps(inputs):
    hidden = np.asarray(inputs["hidden_states"], dtype=np.float32)
    mask = np.asarray(inputs["attention_mask"], dtype=np.float32)
    relation = np.asarray(inputs["relation"], dtype=np.int32)
    wq = np.ascontiguousarray(np.asarray(inputs["Wq"], dtype=np.float32).T)
    wk = np.ascontiguousarray(np.asarray(inputs["Wk"], dtype=np.float32).T)
    wv = np.ascontiguousarray(np.asarray(inputs["Wv"], dtype=np.float32).T)
    bq = np.asarray(inputs["bq"], dtype=np.float32)
    bk = np.asarray(inputs["bk"], dtype=np.float32)
    bv = np.asarray(inputs["bv"], dtype=np.float32)
    remb = np.asarray(inputs["rel_emb"], dtype=np.float32)

    in_maps = []
    for b in range(N_CORES):
        in_maps.append({
            "xT": np.ascontiguousarray(hidden[b].T),
            "wqT": wq, "wkT": wk, "wvT": wv,
            "bq": bq, "bk": bk, "bv": bv,
            "relT": np.ascontiguousarray(relation[b].T),
            "maskv": np.ascontiguousarray(mask[b, 0, 0, :]),
            "relemb": remb,
        })
    return in_maps


LAST_EXEC_NS = None
LAST_RESULTS = None


def kernel(**inputs) -> np.ndarray:
    global LAST_EXEC_NS, LAST_RESULTS
    nc = _get_program()
    in_maps = _make_in_maps(inputs)
    trace = os.environ.get("KERNEL_TRACE", "0") == "1"
    res = run_bass_kernel_spmd(nc, in_maps, list(range(N_CORES)), trace=trace)
    LAST_EXEC_NS = res.exec_time_ns
    LAST_RESULTS = res
    out = np.stack([res.results[b]["out"] for b in range(N_CORES)], axis=0)
    return out.astype(np.float32)


# -------- timing helper: device-resident repeated dispatch --------
def make_bench_fn(inputs):
    """Returns run(M) -> seconds for M back-to-back dispatches (device-resident
    inputs, no donation, block at the end)."""
    import jax
    from jax.sharding import Mesh, PartitionSpec, NamedSharding
    from jax.experimental.shard_map import shard_map
    from concourse import bass2jax
    import concourse.mybir as mb

    nc = _get_program()
    in_maps = _make_in_maps(inputs)
    bass2jax.install_neuronx_cc_hook()

    part_name = nc.partition_id_tensor.name if nc.partition_id_tensor else None
    in_names, out_names, out_avals, zero_outs = [], [], [], []
    for alloc in nc.m.functions[0].allocations:
        if not isinstance(alloc, mb.MemoryLocationSet):
            continue
        name = alloc.memorylocations[0].name
        if alloc.kind == "ExternalInput":
            if name != part_name:
                in_names.append(name)
        elif alloc.kind == "ExternalOutput":
            out_names.append(name)
            shape = tuple(alloc.tensor_shape)
            dtype = mb.dt.np(alloc.dtype)
            out_avals.append(jax.core.ShapedArray(shape, dtype))
            zero_outs.append(np.zeros(shape, dtype))
    n_params = len(in_names)
    all_names = in_names + out_names
    if part_name is not None:
        all_names.append(part_name)

    def _body(*args):
        operands = list(args)
        if part_name is not None:
            operands.append(bass2jax.partition_id_tensor())
        outs = bass2jax._bass_exec_p.bind(
            *operands,
            out_avals=tuple(out_avals),
            in_names=tuple(all_names),
            out_names=tuple(out_names),
            lowering_input_output_aliases=(),
            sim_require_finite=True,
            sim_require_nnan=True,
            nc=nc,
        )
        return tuple(outs)

    devices = jax.devices()[:N_CORES]
    mesh = Mesh(np.asarray(devices), ("core",))
    n_all = n_params + len(out_names)
    sharded = jax.jit(
        shard_map(
            _body, mesh=mesh,
            in_specs=(PartitionSpec("core"),) * n_all,
            out_specs=(PartitionSpec("core"),) * len(out_names),
            check_rep=False,
        ),
        keep_unused=True,
    )
    sh = NamedSharding(mesh, PartitionSpec("core"))
    concat_in = [
        jax.device_put(
            np.concatenate([np.asarray(in_maps[c][nm]) for c in range(N_CORES)], axis=0), sh
        )
        for nm in in_names
    ]
    concat_zeros = [
        jax.device_put(np.zeros((N_CORES * z.shape[0], *z.shape[1:]), z.dtype), sh)
        for z in zero_outs
    ]
    # warmup + compile
    out = sharded(*concat_in, *concat_zeros)
    jax.block_until_ready(out)

    import time

    def run(M):
        t0 = time.perf_counter()
        outs = None
        for _ in range(M):
            outs = sharded(*concat_in, *concat_zeros)
        jax.block_until_ready(outs)
        return time.perf_counter() - t0

    def get_out():
        outs = sharded(*concat_in, *concat_zeros)
        o = np.asarray(outs[0]).reshape(N_CORES, *out_avals[0].shape)
        return o

    run.get_out = get_out
    return run


# -------- simulation helper (single core) for test.py --------
def run_sim_core0(inputs):
    from concourse.bass_interp import CoreSim

    nc = _get_program()
    in_maps = _make_in_maps(inputs)
    sim = CoreSim(nc, trace=False)
    for k, v in in_maps[0].items():
        sim.tensor(k)[:] = v
    sim.simulate(check_with_hw=False)
    return np.array(sim.tensor("out"))


# revision 49
# speedup vs baseline: 2.2670x; 1.4605x over previous
# BERT self-attention with relation bias (Tableformer) on 8 TRN2 NeuronCores.
#
# Strategy (per core = one batch element, pure data parallelism over B=8):
#   - Q^T/K^T/V projections in bf16 on TensorE (inputs pre-transposed host-side,
#     which is pure layout marshalling; all arithmetic runs on-device).
#   - scores computed TRANSPOSED: S^T[k, q] = sum_d K^T[d,k] * Q^T[d,q] so the
#     attention-mask add and softmax plumbing use per-partition (k) bias slots.
#   - softmax without max-subtraction (scores are O(1) here).
#   - relation bias ADDED PRE-EXP: delta_rh = E[r,h] - E[6,h] (normalizing by
#     E[6,h] cancels in softmax).  Two UNCHAINED custom-DVE lookups build
#     Da = delta[rel] for rel in {0,1,2} and Db for rel in {3,4,5} (via a
#     shifted rel-3 plane); each is a single fused 3-entry lookup using the
#     C3 fourth-scalar latch.  The planes depend only on `rel`, NOT on the
#     scores, so the DVE runs flat-out from t~=10us with no upstream stalls.
#   - PE ADDS Da/Db into the scores PSUM via identity-matmul accumulation
#     (psum += I @ D), so no post-exp multiply pass exists at all; ACT's exp
#     reads the fully-biased psum and writes P^T (pt) directly.
#   - ctx^T via a second matmul with P^T as the stationary operand; the softmax
#     denominator comes from a ones-column appended to V (column 64 of V').
#   - final division by the row-sum via ACT Identity with a per-partition
#     reciprocal scale; per-head output slab DMA'd out as soon as the head
#     completes (no tail output flush).
#   - projections are INTERLEAVED into the attention head loop (K/Q block 0
#     first; later blocks + V fill PE gaps), so no serial projection prologue.
import os
import sys
import numpy as np

sys.path.insert(0, "/opt/trn_rl_repo")

import concourse.mybir as mybir  # noqa: E402
from concourse import bass, bacc, tile  # noqa: E402
from concourse.bass_utils import run_bass_kernel_spmd  # noqa: E402
from concourse.dve_ops import DveOp, OPS, CUSTOM_DVE_SPECS, get_dve_sub_opcode  # noqa: E402
from concourse.dve_spec import (  # noqa: E402
    Spec, Src0, Src1, C0, C1, C2, C3, One, Zero, select, eq, lower, _has_src1,
    _spill_c3_to_src1,
)
from concourse.dve_uop import DveOpSpec  # noqa: E402
from concourse.dve_table_gen import dve_ver_for  # noqa: E402

B, S, D, H, HD, NREL = 8, 1024, 1024, 16, 64, 7
N_CORES = 8
P = 128
NT = S // P  # 8 tiles along any 1024 dim
F32 = mybir.dt.float32
BF16 = mybir.dt.bfloat16
I32 = mybir.dt.int32
AF = mybir.ActivationFunctionType
OP = mybir.AluOpType

# ---------------------------------------------------------------------------
# Custom DVE ops.
#   REL_LUT3:  out = (in0==0 ? s0 : in0==1 ? s1 : in0==imm2 ? latch(in1) : 0)
#     A 3-entry additive table lookup; imm2 is 2.0 at every call site, the
#     third value rides the C3 latch (in1 = [P,1] column, read at element 0).
#   REL_LUT2_MUL / REL_LUT2H_MUL / REL_LUT45_MUL:
#     out = (in0==a ? s0 : in0==b ? s1 : 1) * in1 for (a,b) in
#     {(0,1),(2,3),(4,5)} — the chained multiplicative ladder, used for the
#     first two heads where PE is saturated with V/KQ projections.
# ---------------------------------------------------------------------------
_OPS_CACHE = None


def _pin_and_register(op, sp):
    import concourse.dve_ops as _dvo
    OPS.append(op)
    CUSTOM_DVE_SPECS[op.name] = sp
    _dvo._SUB_OPCODE_FOR_NAME[op.name] = _dvo._CUSTOM_DVE_ROW_BASE + len(OPS) - 1
    assert _dvo._SUB_OPCODE_FOR_NAME[op.name] < 0x20
    for ver in ("v3", "v4"):
        try:
            d = DveOpSpec(
                name=op.name,
                opcode=get_dve_sub_opcode(op.name),
                uops=lower(sp, ver=ver),
                rd1_en=_has_src1(sp),
            )
            op.uops_sha[ver] = d.sha(ver)
        except Exception:
            pass


def _register_ops():
    global _OPS_CACHE
    if _OPS_CACHE is not None:
        return _OPS_CACHE
    existing = {op.name: op for op in OPS}
    if "REL_LUT3" in existing:
        _OPS_CACHE = (
            existing["REL_LUT3"], existing["REL_LUT2_MUL"],
            existing["REL_LUT2H_MUL"], existing["REL_LUT45_MUL"],
        )
        return _OPS_CACHE

    body = _spill_c3_to_src1(
        select(eq(Src0, Zero), C0, select(eq(Src0, One), C1, select(eq(Src0, C2), C3, Zero)))
    )

    def _ref3(in0, in1, s0, s1, imm2):
        return np.where(
            in0 == 0, s0, np.where(in0 == 1, s1, np.where(in0 == imm2, in1, np.float32(0.0)))
        ).astype(np.float32)

    sp3 = Spec(body=body, reference=_ref3)
    lut3 = DveOp("REL_LUT3", sp3, subdim=False, uops_sha={})
    _pin_and_register(lut3, sp3)

    two = One + One
    three = two + One
    four = two + two
    five = four + One

    def mk(name, ca, cb, va, vb):
        b = select(eq(Src0, ca), C0, select(eq(Src0, cb), C1, One)) * Src1

        def _ref(in0, in1, s0, s1, imm2, _va=va, _vb=vb):
            return (
                np.where(in0 == _va, s0, np.where(in0 == _vb, s1, np.float32(1.0)))
                * in1
            )

        return name, Spec(body=b, reference=_ref)

    muls = []
    for name, sp in (
        mk("REL_LUT2_MUL", Zero, One, 0, 1),
        mk("REL_LUT2H_MUL", two, three, 2, 3),
        mk("REL_LUT45_MUL", four, five, 4, 5),
    ):
        op = DveOp(name, sp, subdim=False, uops_sha={})
        _pin_and_register(op, sp)
        muls.append(op)

    _OPS_CACHE = (lut3, *muls)
    return _OPS_CACHE


# ---------------------------------------------------------------------------
# Program builder (runs once per process; input-value independent)
# ---------------------------------------------------------------------------
def _build_program():
    lut3, lut01, lut23, lut45 = _register_ops()

    nc = bacc.Bacc(
        "TRN2",
        target_bir_lowering=False,
        debug=False,
        enable_asserts=False,
        num_devices=N_CORES,
    )

    # DRAM I/O (per core)
    xT_d = nc.dram_tensor("xT", [D, S], F32, kind="ExternalInput")       # hidden[b].T  [din, seq]
    wqT_d = nc.dram_tensor("wqT", [D, D], F32, kind="ExternalInput")     # Wq.T [din, dout]
    wkT_d = nc.dram_tensor("wkT", [D, D], F32, kind="ExternalInput")
    wvT_d = nc.dram_tensor("wvT", [D, D], F32, kind="ExternalInput")
    bq_d = nc.dram_tensor("bq", [D], F32, kind="ExternalInput")
    bk_d = nc.dram_tensor("bk", [D], F32, kind="ExternalInput")
    bv_d = nc.dram_tensor("bv", [D], F32, kind="ExternalInput")
    relT_d = nc.dram_tensor("relT", [S, S], I32, kind="ExternalInput")   # relation[b].T  [k, q]
    mask_d = nc.dram_tensor("maskv", [S], F32, kind="ExternalInput")     # attention_mask[b,0,0,:]
    remb_d = nc.dram_tensor("relemb", [NREL, H], F32, kind="ExternalInput")
    out_d = nc.dram_tensor("out", [S, D], F32, kind="ExternalOutput")

    from contextlib import ExitStack

    with tile.TileContext(nc) as tc, ExitStack() as ctx:
        const = ctx.enter_context(tc.tile_pool(name="const", bufs=1))

        # persistent SBUF tensors
        qT = const.tile([P, NT * S], BF16)       # Q^T/8 (+bq/8), dout on partitions
        kT = const.tile([P, NT * S], BF16)       # K^T  (+bk)
        vP = const.tile([P, NT * H * (HD + 1)], BF16)  # V' per seq-block: 16*(64+1)
        rel0 = const.tile([P, NT * S], BF16)     # rel^T as bf16
        rel3 = const.tile([P, NT * S], BF16)     # rel^T - 3
        xT = const.tile([P, NT * S], BF16)       # hidden^T as bf16
        wv = const.tile([P, NT * S], BF16)       # full Wv^T (rhs of V proj)
        mcols = const.tile([P, NT], F32)         # mask column per k-tile
        bqcols = const.tile([P, NT], F32)        # bq/8 column per dout-block
        bkcols = const.tile([P, NT], F32)
        mraw = const.tile([P, 6 * H], F32)       # delta_rh = E[r,h]-E[6,h], col r*16+h
        mprime = const.tile([P, 6 * H], F32)     # exp(delta_rh), for the ladder heads
        ident = const.tile([P, P], BF16)         # 128x128 identity (bias-add lhsT)
        ones_row = const.tile([1, P], F32)       # lhsT for broadcast matmul
        ones_row_bf = const.tile([1, P], BF16)   # bf16 lhsT for rank-1 bias matmul
        bv_row2 = const.tile([1, D], BF16)       # bv as a single-partition row

        with (
            tc.tile_pool(name="prep", bufs=1) as prep,
            tc.tile_pool(name="wrot", bufs=2) as wrot,        # rotating K/Q W-blocks
            tc.tile_pool(name="proj_ps", bufs=2, space="PSUM") as proj_ps,
            tc.tile_pool(name="pt", bufs=2) as ptp,
            tc.tile_pool(name="ex", bufs=2) as exp_pool,
            tc.tile_pool(name="sc_ps", bufs=2, space="PSUM") as sc_psp,
            tc.tile_pool(name="cx_ps", bufs=2, space="PSUM") as cx_psp,
            tc.tile_pool(name="dlt", bufs=4) as dlt,          # Da/Db bias planes
            tc.tile_pool(name="oh", bufs=2) as ohp,           # per-head output slab
            tc.tile_pool(name="rc", bufs=2) as rcp,
        ):
            # ---------------- input DMAs (issue early; Pool dispatch ~1us each)
            def dma_w_block(i):
                """K/Q projection lhsT for dout-block i: W columns i*P..(i+1)*P,
                laid [P din-in-tile, kk*P + col] for the 8 din tiles."""
                wkb = wrot.tile([P, NT * P], BF16, tag="wk")
                wqb = wrot.tile([P, NT * P], BF16, tag="wq")
                nc.gpsimd.dma_start(
                    out=wkb[:].rearrange("p (t c) -> p t c", t=NT),
                    in_=wkT_d[:, i * P:(i + 1) * P].rearrange("(t p) c -> p t c", p=P),
                )
                nc.gpsimd.dma_start(
                    out=wqb[:].rearrange("p (t c) -> p t c", t=NT),
                    in_=wqT_d[:, i * P:(i + 1) * P].rearrange("(t p) c -> p t c", p=P),
                )
                return wkb, wqb

            # The first rel quarter gates the DVE's first Delta lookups —
            # dispatch it before anything else on the Pool queue.  Each rel
            # quarter is DMA-cast int32 -> bf16 (values 0..6 exact) and the
            # shifted rel-3 plane follows on Pool.
            def dma_rel_quarter(lo):
                sl = slice(lo * P, (lo + 2) * P)
                nc.gpsimd.dma_start(
                    out=rel0[:, lo * S:(lo + 2) * S].rearrange("p (t c) -> p t c", t=2),
                    in_=relT_d[sl, :].rearrange("(t p) c -> p t c", p=P),
                )
                nc.gpsimd.tensor_scalar_add(
                    rel3[:, lo * S:(lo + 2) * S],
                    rel0[:, lo * S:(lo + 2) * S],
                    -3.0,
                )

            # identity-matrix ingredients first on the Pool queue (no deps;
            # the ident tensor gates the DVE instruction stream)
            riota = prep.tile([P, P], F32)
            piota = prep.tile([P, 1], F32)
            nc.gpsimd.iota(riota[:], [[1, P]], channel_multiplier=0,
                           allow_small_or_imprecise_dtypes=True)
            nc.gpsimd.iota(piota[:], [[0, 1]], channel_multiplier=1,
                           allow_small_or_imprecise_dtypes=True)

            dma_rel_quarter(0)
            w0 = dma_w_block(0)

            # bulk loads, 2 batched DMAs each (split for DMA-queue parallelism)
            for lo in (0, NT // 2):
                sl = slice(lo * P, (lo + NT // 2) * P)
                nc.gpsimd.dma_start(
                    out=xT[:, lo * S:(lo + NT // 2) * S].rearrange("p (t c) -> p t c", t=NT // 2),
                    in_=xT_d[sl, :].rearrange("(t p) c -> p t c", p=P),
                )
            for lo in (2, 4, 6):
                dma_rel_quarter(lo)
            for lo in (0, NT // 2):
                sl = slice(lo * P, (lo + NT // 2) * P)
                nc.gpsimd.dma_start(
                    out=wv[:, lo * S:(lo + NT // 2) * S].rearrange("p (t c) -> p t c", t=NT // 2),
                    in_=wvT_d[sl, :].rearrange("(t p) c -> p t c", p=P),
                )

            # ---------------- constants prep ----------------
            nc.sync.dma_start(out=mcols[:], in_=mask_d[:].rearrange("(t p) -> p t", p=P))
            nc.sync.dma_start(out=bqcols[:], in_=bq_d[:].rearrange("(t p) -> p t", p=P))
            nc.sync.dma_start(out=bkcols[:], in_=bk_d[:].rearrange("(t p) -> p t", p=P))
            nc.vector.tensor_scalar_mul(bqcols[:], bqcols[:], 0.125)

            nc.vector.memset(ones_row[:], 1.0)

            # rel_emb broadcast to all partitions: [1,112] -> psum [128,112]
            remb_row = prep.tile([1, NREL * H], F32)
            nc.sync.dma_start(
                out=remb_row[:], in_=remb_d[:].rearrange("r h -> (r h)").rearrange("(o n) -> o n", o=1)
            )
            mb_ps = proj_ps.tile([P, 512], F32, tag="pps")
            nc.tensor.matmul(mb_ps[:, 0:NREL * H], ones_row[:], remb_row[:])
            mb_sb = prep.tile([P, NREL * H], F32)
            nc.vector.tensor_copy(mb_sb[:], mb_ps[:, 0:NREL * H])
            # delta_r = E[r,:] - E[6,:] for r=0..5
            for r in range(6):
                nc.vector.tensor_tensor(
                    mraw[:, r * H:(r + 1) * H],
                    mb_sb[:, r * H:(r + 1) * H],
                    mb_sb[:, 6 * H:7 * H],
                    OP.subtract,
                )
            nc.scalar.activation(mprime[:], mraw[:], AF.Exp)

            # identity matrix for the PSUM bias-add matmuls:
            # riota[p, c] = c ; piota[p, 0] = p ; I = (riota == piota)
            nc.vector.tensor_scalar(ident[:], riota[:], piota[:], None, OP.is_equal)

            nc.gpsimd.dma_start(out=bv_row2[:], in_=bv_d[:].rearrange("(o d) -> o d", o=1))
            nc.vector.memset(ones_row_bf[:], 1.0)

            # V' gets ones in column 64 of each head slot (denominator column)
            nc.gpsimd.memset(vP[:], 1.0)

            # ---------------- emission helpers ----------------
            def emit_kq_block(i, wpair):
                """Project K then Q for dout-block i (heads 2i, 2i+1)."""
                wkb, wqb = wpair
                for which in ("k", "q"):
                    wt = wkb if which == "k" else wqb
                    dst = kT if which == "k" else qT
                    bias_cols = bkcols if which == "k" else bqcols
                    scale = 1.0 if which == "k" else 0.125
                    for j in range(2):
                        ps = proj_ps.tile([P, 512], F32, tag="pps")
                        for kk in range(NT):
                            nc.tensor.matmul(
                                ps[:],
                                wt[:, kk * P:(kk + 1) * P],
                                xT[:, kk * S + j * 512: kk * S + (j + 1) * 512],
                                start=(kk == 0),
                                stop=(kk == NT - 1),
                            )
                        nc.scalar.activation(
                            dst[:, i * S + j * 512: i * S + (j + 1) * 512],
                            ps[:], AF.Identity,
                            bias=bias_cols[:, i:i + 1], scale=scale,
                        )

            def emit_v_block(sb):
                """Project V for seq-block sb into vP (natural layout + ones col)."""
                for j in range(2):
                    ps = proj_ps.tile([P, 512], F32, tag="pps")
                    for kk in range(NT):
                        nc.tensor.matmul(
                            ps[:],
                            xT[:, kk * S + sb * P: kk * S + (sb + 1) * P],
                            wv[:, kk * S + j * 512: kk * S + (j + 1) * 512],
                            start=(kk == 0),
                            stop=False,
                        )
                    # + bv via a rank-1 accumulating matmul (ones column x bv row)
                    nc.tensor.matmul(
                        ps[:],
                        ones_row_bf[:],
                        bv_row2[:, j * 512:(j + 1) * 512],
                        start=False,
                        stop=True,
                    )
                    vslot = vP[:, sb * H * 65 + j * 8 * 65: sb * H * 65 + (j + 1) * 8 * 65].rearrange(
                        "p (h e) -> p h e", h=8
                    )[:, :, 0:HD]
                    nc.scalar.activation(
                        vslot,
                        ps[:].rearrange("p (h e) -> p h e", h=8),
                        AF.Copy,
                    )

            def emit_ctx(h, pt):
                oh = ohp.tile([P, NT * HD], BF16, tag="oh")
                for qb in range(NT):
                    cps = cx_psp.tile([P, HD + 1], F32, tag="cps")
                    for kb in range(NT):
                        nc.tensor.matmul(
                            cps[:],
                            pt[:, kb * S + qb * P: kb * S + (qb + 1) * P],
                            vP[:, kb * H * 65 + h * 65: kb * H * 65 + (h + 1) * 65],
                            start=(kb == 0),
                            stop=(kb == NT - 1),
                        )
                    rc = rcp.tile([P, 1], F32, tag="rc")
                    nc.vector.reciprocal(rc[:], cps[:, HD:HD + 1])
                    nc.scalar.activation(
                        oh[:, qb * HD:(qb + 1) * HD],
                        cps[:, 0:HD], AF.Identity, bias=0.0, scale=rc[:],
                    )
                # stream this head's output columns to DRAM
                nc.gpsimd.dma_start(
                    out=out_d[:, h * HD:(h + 1) * HD].rearrange("(t p) c -> p t c", p=P),
                    in_=oh[:].rearrange("p (t c) -> p t c", t=NT),
                )

            # ---------------- interleaved projections + attention ----------
            emit_kq_block(0, w0)

            N_LADDER = 1  # heads on the post-exp multiplicative-ladder path

            def emit_delta(h, kb2):
                """Bias planes for 2 k-tiles: depend only on rel + rel_emb, so
                the DVE never waits on the score pipeline."""
                da = dlt.tile([P, 2 * S], BF16, tag="da")
                db = dlt.tile([P, 2 * S], BF16, tag="db")
                sl2 = slice(kb2 * 2 * S, (kb2 * 2 + 2) * S)
                nc.vector._custom_dve(
                    lut3, out=da[:], in0=rel0[:, sl2],
                    in1=mraw[:, 2 * H + h: 2 * H + h + 1],
                    s0=mraw[:, 0 * H + h: 0 * H + h + 1],
                    s1=mraw[:, 1 * H + h: 1 * H + h + 1],
                    imm2=2.0,
                )
                nc.vector._custom_dve(
                    lut3, out=db[:], in0=rel3[:, sl2],
                    in1=mraw[:, 5 * H + h: 5 * H + h + 1],
                    s0=mraw[:, 3 * H + h: 3 * H + h + 1],
                    s1=mraw[:, 4 * H + h: 4 * H + h + 1],
                    imm2=2.0,
                )
                return da, db

            # head N_LADDER's bias planes are emitted up front so the DVE
            # starts producing them the moment rel0/rel3/mraw land; head
            # N_LADDER+1's are pre-emitted after ladder head 0 (below) so the
            # DVE never waits for the trickle of ladder-head exps.
            stash = {
                N_LADDER: [emit_delta(N_LADDER, kb2) for kb2 in range(NT // 2)]
            }

            pend = []
            wnext = None
            for h in range(H):
                off = (h % 2) * HD
                hc = h // 2
                # prefetch the next K/Q W-block one head-pair ahead; the
                # projection itself is emitted AFTER this head's score groups
                # so the exp stream (which feeds the DVE) is never delayed.
                if h % 2 == 0 and hc + 1 < NT:
                    wnext = dma_w_block(hc + 1)

                pt = ptp.tile([P, NT * S], BF16, tag="pt")
                if h < N_LADDER:
                    # ladder path: plain scores, exp, then 3 chained
                    # lookup-multiply ops.  No identity matmuls — PE is
                    # saturated with V/KQ projections during these heads,
                    # while the DVE would otherwise sit idle.
                    for kb2 in range(NT // 2):
                        ex = exp_pool.tile([P, 2 * S], BF16, tag="ex")
                        for kh in range(2):
                            kb = kb2 * 2 + kh
                            ps = sc_psp.tile([P, S], F32, tag="scps")
                            for j in range(2):
                                nc.tensor.matmul(
                                    ps[:, j * 512:(j + 1) * 512],
                                    kT[off:off + HD, hc * S + kb * P: hc * S + (kb + 1) * P],
                                    qT[off:off + HD, hc * S + j * 512: hc * S + (j + 1) * 512],
                                )
                            nc.scalar.activation(
                                ex[:, kh * S:(kh + 1) * S], ps[:], AF.Exp,
                                bias=mcols[:, kb:kb + 1], scale=1.0,
                            )
                        sl2 = slice(kb2 * 2 * S, (kb2 * 2 + 2) * S)
                        t1 = exp_pool.tile([P, 2 * S], BF16, tag="t1", bufs=1)
                        t2 = exp_pool.tile([P, 2 * S], BF16, tag="t2", bufs=1)
                        nc.vector._custom_dve(
                            lut01, out=t1[:], in0=rel0[:, sl2], in1=ex[:],
                            s0=mprime[:, 0 * H + h: 0 * H + h + 1],
                            s1=mprime[:, 1 * H + h: 1 * H + h + 1],
                        )
                        nc.vector._custom_dve(
                            lut23, out=t2[:], in0=rel0[:, sl2], in1=t1[:],
                            s0=mprime[:, 2 * H + h: 2 * H + h + 1],
                            s1=mprime[:, 3 * H + h: 3 * H + h + 1],
                        )
                        nc.vector._custom_dve(
                            lut45, out=pt[:, sl2], in0=rel0[:, sl2], in1=t2[:],
                            s0=mprime[:, 4 * H + h: 4 * H + h + 1],
                            s1=mprime[:, 5 * H + h: 5 * H + h + 1],
                        )
                else:
                    hstash = stash.pop(h, None)
                    for kb2 in range(NT // 2):
                        if hstash is not None:
                            da, db = hstash[kb2]
                        else:
                            da, db = emit_delta(h, kb2)
                        for kh in range(2):
                            kb = kb2 * 2 + kh
                            ps = sc_psp.tile([P, S], F32, tag="scps")
                            for j in range(2):
                                nc.tensor.matmul(
                                    ps[:, j * 512:(j + 1) * 512],
                                    kT[off:off + HD, hc * S + kb * P: hc * S + (kb + 1) * P],
                                    qT[off:off + HD, hc * S + j * 512: hc * S + (j + 1) * 512],
                                    start=True, stop=False,
                                )
                                nc.tensor.matmul(
                                    ps[:, j * 512:(j + 1) * 512],
                                    ident[:],
                                    da[:, kh * S + j * 512: kh * S + (j + 1) * 512],
                                    start=False, stop=False,
                                )
                                nc.tensor.matmul(
                                    ps[:, j * 512:(j + 1) * 512],
                                    ident[:],
                                    db[:, kh * S + j * 512: kh * S + (j + 1) * 512],
                                    start=False, stop=True,
                                )
                            nc.scalar.activation(
                                pt[:, kb * S:(kb + 1) * S], ps[:], AF.Exp,
                                bias=mcols[:, kb:kb + 1], scale=1.0,
                            )

                if h % 2 == 1 and wnext is not None:
                    emit_kq_block(h // 2 + 1, wnext)
                    wnext = None

                # V projections fill the PE gaps during the first two heads;
                # all 8 blocks are emitted before ctx(0).
                if h == 0:
                    for sb in range(6):
                        emit_v_block(sb)
                elif h == 1:
                    for sb in range(6, NT):
                        emit_v_block(sb)

                # ctx pipelined one head behind: PE emits scores(h+1)
                # before ctx(h) would otherwise block it.
                pend.append((h, pt))
                if len(pend) > 1:
                    emit_ctx(*pend.pop(0))

            for e in pend:
                emit_ctx(*e)

    nc.compile()
    return nc


_PROGRAM = None


def _get_program():
    global _PROGRAM
    if _PROGRAM is None:
        _PROGRAM = _build_program()
    return _PROGRAM


def _make_in_maps(inputs):
    hidden = np.asarray(inputs["hidden_states"], dtype=np.float32)
    mask = np.asarray(inputs["attention_mask"], dtype=np.float32)
    relation = np.asarray(inputs["relation"], dtype=np.int32)
    wq = np.ascontiguousarray(np.asarray(inputs["Wq"], dtype=np.float32).T)
    wk = np.ascontiguousarray(np.asarray(inputs["Wk"], dtype=np.float32).T)
    wv = np.ascontiguousarray(np.asarray(inputs["Wv"], dtype=np.float32).T)
    bq = np.asarray(inputs["bq"], dtype=np.float32)
    bk = np.asarray(inputs["bk"], dtype=np.float32)
    bv = np.asarray(inputs["bv"], dtype=np.float32)
    remb = np.asarray(inputs["rel_emb"], dtype=np.float32)

    in_maps = []
    for b in range(N_CORES):
        in_maps.append({
            "xT": np.ascontiguousarray(hidden[b].T),
            "wqT": wq, "wkT": wk, "wvT": wv,
            "bq": bq, "bk": bk, "bv": bv,
            "relT": np.ascontiguousarray(relation[b].T),
            "maskv": np.ascontiguousarray(mask[b, 0, 0, :]),
            "relemb": remb,
        })
    return in_maps


LAST_EXEC_NS = None
LAST_RESULTS = None


def kernel(**inputs) -> np.ndarray:
    global LAST_EXEC_NS, LAST_RESULTS
    nc = _get_program()
    in_maps = _make_in_maps(inputs)
    trace = os.environ.get("KERNEL_TRACE", "0") == "1"
    res = run_bass_kernel_spmd(nc, in_maps, list(range(N_CORES)), trace=trace)
    LAST_EXEC_NS = res.exec_time_ns
    LAST_RESULTS = res
    out = np.stack([res.results[b]["out"] for b in range(N_CORES)], axis=0)
    return out.astype(np.float32)


# -------- timing helper: device-resident repeated dispatch --------
def make_bench_fn(inputs):
    """Returns run(M) -> seconds for M back-to-back dispatches (device-resident
    inputs, no donation, block at the end)."""
    import jax
    from jax.sharding import Mesh, PartitionSpec, NamedSharding
    from jax.experimental.shard_map import shard_map
    from concourse import bass2jax
    import concourse.mybir as mb

    nc = _get_program()
    in_maps = _make_in_maps(inputs)
    bass2jax.install_neuronx_cc_hook()

    part_name = nc.partition_id_tensor.name if nc.partition_id_tensor else None
    in_names, out_names, out_avals, zero_outs = [], [], [], []
    for alloc in nc.m.functions[0].allocations:
        if not isinstance(alloc, mb.MemoryLocationSet):
            continue
        name = alloc.memorylocations[0].name
        if alloc.kind == "ExternalInput":
            if name != part_name:
                in_names.append(name)
        elif alloc.kind == "ExternalOutput":
            out_names.append(name)
            shape = tuple(alloc.tensor_shape)
            dtype = mb.dt.np(alloc.dtype)
            out_avals.append(jax.core.ShapedArray(shape, dtype))
            zero_outs.append(np.zeros(shape, dtype))
    n_params = len(in_names)
    all_names = in_names + out_names
    if part_name is not None:
        all_names.append(part_name)

    def _body(*args):
        operands = list(args)
        if part_name is not None:
            operands.append(bass2jax.partition_id_tensor())
        outs = bass2jax._bass_exec_p.bind(
            *operands,
            out_avals=tuple(out_avals),
            in_names=tuple(all_names),
            out_names=tuple(out_names),
            lowering_input_output_aliases=(),
            sim_require_finite=True,
            sim_require_nnan=True,
            nc=nc,
        )
        return tuple(outs)

    devices = jax.devices()[:N_CORES]
    mesh = Mesh(np.asarray(devices), ("core",))
    n_all = n_params + len(out_names)
    sharded = jax.jit(
        shard_map(
            _body, mesh=mesh,
            in_specs=(PartitionSpec("core"),) * n_all,
            out_specs=(PartitionSpec("core"),) * len(out_names),
            check_rep=False,
        ),
        keep_unused=True,
    )
    sh = NamedSharding(mesh, PartitionSpec("core"))
    concat_in = [
        jax.device_put(
            np.concatenate([np.asarray(in_maps[c][nm]) for c in range(N_CORES)], axis=0), sh
        )
        for nm in in_names
    ]
    concat_zeros = [
        jax.device_put(np.zeros((N_CORES * z.shape[0], *z.shape[1:]), z.dtype), sh)
        for z in zero_outs
    ]
    # warmup + compile
    out = sharded(*concat_in, *concat_zeros)
    jax.block_until_ready(out)

    import time

    def run(M):
        t0 = time.perf_counter()
        outs = None
        for _ in range(M):
            outs = sharded(*concat_in, *concat_zeros)
        jax.block_until_ready(outs)
        return time.perf_counter() - t0

    def get_out():
        outs = sharded(*concat_in, *concat_zeros)
        o = np.asarray(outs[0]).reshape(N_CORES, *out_avals[0].shape)
        return o

    run.get_out = get_out
    return run


# -------- simulation helper (single core) for test.py --------
def run_sim_core0(inputs):
    from concourse.bass_interp import CoreSim

    nc = _get_program()
    in_maps = _make_in_maps(inputs)
    sim = CoreSim(nc, trace=False)
    for k, v in in_maps[0].items():
        sim.tensor(k)[:] = v
    sim.simulate(check_with_hw=False)
    return np.array(sim.tensor("out"))
